# revision 1
# baseline (speedup 1.0000x reference)
"""Multi-head attention Bass kernel for Trainium2 (8 NeuronCores).

Problem: B=8, T=2048, C=256, H=8, D=32 MHA (dense, full softmax over T).
Sharding: data-parallel over batch -- core b computes batch b end-to-end,
no collectives.  Weights are replicated; per-core x slice is [T, C].

Per-core dataflow:
  1. x [T,C] -> xT [C,T] via PE transposes (fp32), rounded to bf16.
  2. qT/kT [D,T] per head (packed 4 heads per [128,T] tile via column
     tiling, bf16) and v [T,D] per head with an appended ones-column
     (v_ext [T,33], bf16).  wq is pre-scaled by 1/sqrt(C)=1/16 host-side
     (exact, power of two), so scores come out pre-scaled.
  3. Scores computed TRANSPOSED: weiT[s,t] = k[s]*q[t] per head (bf16
     matmuls, fp32 PSUM accumulate); ScalarE exp() PSUM->SBUF(bf16) --
     the critical path: 256 instrs of [128,1024].
  4. AV: outT_ext[33,t] = v_ext.T @ expw accumulated over s-chunks in
     PSUM (bf16 matmuls); row 32 is the softmax denominator (ones
     column).  Two heads packed per PSUM tile via column tiling at
     partitions 0 and 64.
  5. Normalize: reciprocal_approx of the denominator rows, broadcast
     across 32 partitions with a K=1 fp32 PE matmul against ones, then
     a DVE multiply into fp32r nout tiles.
  6. Projection (fp32r, full-array): res = sum_pairs noutT_p.T @
     wproj_p + bias, with wproj zero-padded host-side to the pair
     layout (heads at partition rows 0-31 / 64-95).

dtypes: tiled matmuls (scores/AV/QKV) are bf16 (fp32r cannot be
combined with tile_position on trn2 -- walrus rejects it); the
projection is fp32r; transposes and the K=1 broadcast are fp32.
"""

import numpy as np
from contextlib import ExitStack

import concourse.bass as bass
import concourse.bacc as bacc
import concourse.mybir as mybir
import concourse.tile as tile
from concourse.bass_utils import run_bass_kernel_spmd
from concourse.masks import make_identity

B, T, C, H, D = 8, 2048, 256, 8, 32
P = 128
NT = T // P  # 16 chunks of 128 along t / s
F32 = mybir.dt.float32
F32R = mybir.dt.float32r
BF16 = mybir.dt.bfloat16
EXP = mybir.ActivationFunctionType.Exp
N_CORES = 8
E = D + 1  # 33: v columns + ones column


def _body(nc, tc, ctx, x_d, wq_d, wk_d, wv_d, wp_d, bias_d, out_d):
    const = ctx.enter_context(tc.tile_pool(name="const", bufs=1))
    big = ctx.enter_context(tc.tile_pool(name="big", bufs=1))

    ident = const.tile([P, P], F32)
    make_identity(nc, ident)

    wq_sb = const.tile([P, 2, C], BF16)
    wk_sb = const.tile([P, 2, C], BF16)
    wv_sb = const.tile([P, 2, C], BF16)
    wp_sb = const.tile([P, 4, C], F32R)
    bias_sb = const.tile([P, C], F32)
    nc.sync.dma_start(out=bias_sb, in_=bias_d)
    ones_sb = const.tile([P, D], F32)
    nc.vector.memset(ones_sb, 1.0)
    warm = const.tile([P, 1], F32)
    nc.scalar.activation(out=warm, in_=ones_sb[:, 0:1], func=EXP)

    xT = [big.tile([P, T], BF16, name=f"xT{i}") for i in range(2)]
    qT = [big.tile([P, T], BF16, name=f"qT{i}") for i in range(2)]
    kT = [big.tile([P, T], BF16, name=f"kT{i}") for i in range(2)]
    v_sb = big.tile([P, NT, E * H], BF16)
    nc.vector.memset(v_sb, 1.0)  # ones columns survive; v overwrites the rest
    nout = [big.tile([P, T], F32R, name=f"nout{i}") for i in range(4)]

    # ---- Phase 1: loads (into fresh regions only -- DIRECT2D DMAs support
    # a single sync wait), zero nout, build xT via PE transposes ----------
    with tc.tile_pool(name="xload", bufs=1) as xpool, \
         tc.tile_pool(name="pt", bufs=4, space="PSUM") as pt:
        x_sb = xpool.tile([P, NT, C], F32)
        x_r = x_d.rearrange("(n p) c -> p n c", p=P)
        for n in range(NT):  # chunked so transposes start on first arrival
            nc.sync.dma_start(out=x_sb[:, n], in_=x_r[:, n])
        with tc.tile_pool(name="wstage", bufs=4) as wstage:
            for w_sb, w_d, nk in ((wq_sb, wq_d, 2), (wk_sb, wk_d, 2),
                                  (wv_sb, wv_d, 2), (wp_sb, wp_d, 4)):
                w_st = wstage.tile([P, nk, C], F32, tag="wst", name="w_st")
                if nk == 2:
                    nc.sync.dma_start(
                        out=w_st, in_=w_d.rearrange("(k p) c -> p k c", p=P))
                else:
                    nc.sync.dma_start(out=w_st, in_=w_d.rearrange("q p c -> p q c"))
                nc.vector.tensor_copy(w_sb, w_st)
            # fp32r memsets are not encodable; stage zeros through fp32
            zst = wstage.tile([P, T], F32, tag="zst", name="zst")
            nc.vector.memset(zst, 0.0)
            for t_ in nout:
                nc.vector.tensor_copy(t_, zst)  # rows 32-63 / 96-127 stay 0
        for n in range(NT):
            for cc in range(2):
                tp = pt.tile([P, P], F32, tag="tp", name="tp")
                nc.tensor.transpose(tp, x_sb[:, n, cc * P:(cc + 1) * P], ident)
                nc.vector.tensor_copy(xT[cc][:, n * P:(n + 1) * P], tp)

    # ---- Phase 2b: qT / kT (4 heads packed per [128,T] tile) ------------
    with tc.tile_pool(name="pq", bufs=2, space="PSUM") as pq:
        for w_sb, dest in ((wq_sb, qT), (wk_sb, kT)):
            for g in range(2):
                qp = pq.tile([P, T], F32, tag="qp", name="qp")
                for j in range(4):
                    h = 4 * g + j
                    for ts in range(4):
                        for cc in range(2):
                            nc.tensor.matmul(
                                qp[D * j:D * (j + 1), 512 * ts:512 * (ts + 1)],
                                lhsT=w_sb[:, cc, D * h:D * (h + 1)],
                                rhs=xT[cc][:, 512 * ts:512 * (ts + 1)],
                                start=(cc == 0), stop=(cc == 1),
                                tile_position=(0, D * j))
                nc.vector.tensor_copy(dest[g], qp)

    # ---- Phase 2a: v projection (all 8 heads per matmul, N=256) ---------
    with tc.tile_pool(name="pv", bufs=2, space="PSUM") as pv:
        for n in range(NT):
            vp = pv.tile([P, C], F32, tag="vp", name="vp")
            for cc in range(2):
                nc.tensor.matmul(
                    vp,
                    lhsT=xT[cc][:, n * P:(n + 1) * P],
                    rhs=wv_sb[:, cc, :],
                    start=(cc == 0), stop=(cc == 1))
            nc.vector.tensor_copy(
                v_sb[:, n].rearrange("p (h e) -> p h e", e=E)[:, :, 0:D],
                vp.rearrange("p (h d) -> p h d", d=D))


    # ---- Phase 3: attention, head pairs ---------------------------------
    with tc.tile_pool(name="scp", bufs=2, space="PSUM") as scp, \
         tc.tile_pool(name="avp", bufs=1, space="PSUM") as avp, \
         tc.tile_pool(name="expp", bufs=3) as expp, \
         tc.tile_pool(name="nrm", bufs=2) as nrm:
        for pair in range(4):
            g = pair // 2
            hA, hB = 2 * pair, 2 * pair + 1
            av = avp.tile([P, T], F32, tag="av", name="av")
            for s in range(NT):
                exs = []
                for h in (hA, hB):
                    j = h % 4
                    for half in range(2):
                        sc = scp.tile([P, 1024], F32, tag="sc", name="sc")
                        for ts in range(2):
                            tofs = 1024 * half + 512 * ts
                            nc.tensor.matmul(
                                sc[:, 512 * ts:512 * (ts + 1)],
                                lhsT=kT[g][D * j:D * (j + 1), P * s:P * (s + 1)],
                                rhs=qT[g][D * j:D * (j + 1), tofs:tofs + 512],
                                start=True, stop=True,
                                tile_position=(D * j, 0))
                        ex = expp.tile([P, 1024], BF16, tag=f"ex{h - hA}{half}",
                                       name="ex")
                        nc.scalar.activation(out=ex, in_=sc, func=EXP)
                        exs.append((h, half, ex))
                for h, half, ex in exs:
                    col = 0 if h == hA else 64
                    for ts in range(2):
                        tofs = 1024 * half + 512 * ts
                        # A and B share banks at different partition ranges;
                        # per-element has_written makes that safe on HW, but
                        # the sim's bank-granular group tracker would flag it.
                        nc.tensor.matmul(
                            av[col:col + E, tofs:tofs + 512],
                            lhsT=v_sb[:, s, E * h:E * h + E],
                            rhs=ex[:, 512 * ts:512 * (ts + 1)],
                            start=(s == 0), stop=(s == NT - 1),
                            tile_position=(0, col),
                            skip_group_check=True)
            # normalize: recip of denominator rows (32 for A, 96 for B),
            # broadcast to 32 partitions via K=1 fp32 matmul against ones.
            recip = nrm.tile([P, T], F32, tag="recip", name="recip")
            for row in (D, 64 + D):
                # reciprocal_approx_fast (custom DVE) gives garbage on HW via
                # this path; the stock iterative-divide reciprocal is exact.
                nc.vector.reciprocal(
                    out=recip[row:row + 1, :], in_=av[row:row + 1, :])
            for half in range(2):
                bc = scp.tile([P, 1024], F32, tag="sc", name="bc")
                for colofs, row in ((0, D), (64, 64 + D)):
                    rg = (row // 32) * 32
                    for ts in range(2):
                        tofs = 1024 * half + 512 * ts
                        nc.tensor.matmul(
                            bc[colofs:colofs + D, 512 * ts:512 * (ts + 1)],
                            lhsT=ones_sb[row:row + 1, 0:D],
                            rhs=recip[row:row + 1, tofs:tofs + 512],
                            start=True, stop=True,
                            tile_position=(rg, colofs),
                            skip_group_check=True)
                bc_sb = nrm.tile([P, 1024], F32, tag="bc", name="bc_sb")
                nc.vector.tensor_copy(bc_sb[0:D, :], bc[0:D, :])
                nc.vector.tensor_copy(bc_sb[64:64 + D, :], bc[64:64 + D, :])
                tofs0 = 1024 * half
                nc.vector.tensor_mul(
                    nout[pair][0:D, tofs0:tofs0 + 1024],
                    av[0:D, tofs0:tofs0 + 1024],
                    bc_sb[0:D, :])
                nc.vector.tensor_mul(
                    nout[pair][64:64 + D, tofs0:tofs0 + 1024],
                    av[64:64 + D, tofs0:tofs0 + 1024],
                    bc_sb[64:64 + D, :])

    # ---- Phase 4: output projection + bias (fp32r, full array) ----------
    with tc.tile_pool(name="prp", bufs=2, space="PSUM") as prp, \
         tc.tile_pool(name="resp", bufs=3) as resp:
        out_r = out_d.rearrange("(n p) c -> n p c", p=P)
        for n in range(NT):
            rp = prp.tile([P, C], F32, tag="rp", name="rp")
            for q in range(4):
                nc.tensor.matmul(
                    rp,
                    lhsT=nout[q][:, P * n:P * (n + 1)],
                    rhs=wp_sb[:, q, :],
                    start=(q == 0), stop=(q == 3))
            res = resp.tile([P, C], F32, tag="res", name="res")
            nc.vector.tensor_add(res, rp, bias_sb)
            nc.sync.dma_start(out=out_r[n], in_=res)


def build_nc():
    nc = bacc.Bacc("TRN2", debug=False, num_devices=N_CORES)
    x_d = nc.dram_tensor("x", [T, C], F32, kind="ExternalInput")
    wq_d = nc.dram_tensor("wq", [C, C], F32, kind="ExternalInput")
    wk_d = nc.dram_tensor("wk", [C, C], F32, kind="ExternalInput")
    wv_d = nc.dram_tensor("wv", [C, C], F32, kind="ExternalInput")
    wp_d = nc.dram_tensor("wp", [4, P, C], F32, kind="ExternalInput")
    bias_d = nc.dram_tensor("bias", [P, C], F32, kind="ExternalInput")
    out_d = nc.dram_tensor("out", [T, C], F32, kind="ExternalOutput")
    with tile.TileContext(nc) as tc:
        with ExitStack() as ctx:
            _body(nc, tc, ctx, x_d.ap(), wq_d.ap(), wk_d.ap(), wv_d.ap(),
                  wp_d.ap(), bias_d.ap(), out_d.ap())
    nc.compile()
    return nc


def prep_inputs(x, wq, wk, wv, wproj, bproj):
    """Host-side reformatting of the full inputs into per-core input maps."""
    f = np.float32
    # [H,C,D] -> [C, H*D]; wq additionally pre-scaled by 1/sqrt(C) (exact).
    wq2 = np.ascontiguousarray(
        np.transpose(np.asarray(wq, f), (1, 0, 2)).reshape(C, H * D)) * f(1.0 / 16.0)
    wk2 = np.ascontiguousarray(
        np.transpose(np.asarray(wk, f), (1, 0, 2)).reshape(C, H * D))
    wv2 = np.ascontiguousarray(
        np.transpose(np.asarray(wv, f), (1, 0, 2)).reshape(C, H * D))
    # wproj [H*D, C] -> 4 pair-chunks padded to 128 rows:
    # rows 0-31 <- head 2p, rows 64-95 <- head 2p+1, rest zero.
    wp4 = np.zeros((4, P, C), f)
    wproj = np.asarray(wproj, f)
    for p in range(4):
        wp4[p, 0:D] = wproj[64 * p: 64 * p + D]
        wp4[p, 64:64 + D] = wproj[64 * p + D: 64 * p + 2 * D]
    bias128 = np.ascontiguousarray(
        np.broadcast_to(np.asarray(bproj, f), (P, C)))
    x = np.asarray(x, f)
    in_maps = []
    for b in range(N_CORES):
        in_maps.append({
            "x": np.ascontiguousarray(x[b]),
            "wq": wq2, "wk": wk2, "wv": wv2,
            "wp": wp4, "bias": bias128,
        })
    return in_maps


def kernel(x, wq, wk, wv, wproj, bproj, _nc=None):
    in_maps = prep_inputs(x, wq, wk, wv, wproj, bproj)
    nc = _nc if _nc is not None else build_nc()
    res = run_bass_kernel_spmd(nc, in_maps, list(range(N_CORES)))
    return np.stack([r["out"] for r in res.results], axis=0)



# revision 49
# speedup vs baseline: 154.8301x; 154.8301x over previous
"""Multi-head attention Bass kernel for Trainium2 (8 NeuronCores).

Problem: B=8, T=2048, C=256, H=8, D=32 MHA (dense, full softmax over T).
Sharding: data-parallel over batch -- core b computes batch b end-to-end,
no collectives.  Weights are replicated; per-core x slice is [T, C].

Per-core dataflow (v2 -- ScalarE-exp-bound design):
  1. x [T,C] -> xT [C,T] via PE transposes (fp32), rounded to bf16.
  2. qT/kT [D,T] per head (packed 4 heads per [128,T] tile, bf16) and
     v [T,D] per head with an appended ones-column (v_ext [T,33], bf16).
     wq pre-scaled by 1/sqrt(C)=1/16 host-side (exact).
  3. Scores TRANSPOSED: weiT[s,t] = k[s]*q[t] per head-pair.  Inner unit
     = (s-chunk, t-quarter): 2 score MMs (heads A,B at adjacent PE
     row-groups) -> one ScalarE exp ACT of [128,2,512] (FD=1024,
     PSUM->SBUF bf16) -> 2 AV MMs (tile_position cols 0/64, PSUM
     accumulate over s; ones-column gives the softmax denominator at
     rows 32/96).  Units software-pipelined one ahead so ScalarE (the
     roofline engine: 256 ACTs ~ 294us) never starves; PSUM = av 4
     banks + 2x sc 2 banks = 8.
  4. Normalize (off critical path, overlapped with next pair): DVE
     copies av[0:33]/[64:97] -> SBUF (frees av banks), DVE
     reciprocal_approx_fast on the denominator rows, DMA broadcasts the
     recip rows across partitions 0-31/64-95 (free engine), one DVE
     tensor mult -> nout bf16 (rows 33-63 stay zero; row 32/96 ~ 1.0
     but projection weights there are zero-padded).
  5. Projection TRANSPOSED: resT[c,t] = sum_p wp_p.T @ nout_p + bias,
     bias folded in as a K=1 matmul against a ones row.  Output written
     to DRAM as [2, 128, T] (c-half, c, t); host transposes to [T, C].

dtypes: all matmuls bf16 (fp32 PSUM accumulate); exp ACT fp32->bf16.
"""

import numpy as np
from contextlib import ExitStack

import concourse.bass as bass
import concourse.bacc as bacc
import concourse.mybir as mybir
import concourse.tile as tile
from concourse.bass_utils import run_bass_kernel_spmd
from concourse.masks import make_identity

B, T, C, H, D = 8, 2048, 256, 8, 32
P = 128
NT = T // P  # 16 chunks of 128 along t / s
F32 = mybir.dt.float32
BF16 = mybir.dt.bfloat16
I16 = mybir.dt.int16
EXP = mybir.ActivationFunctionType.Exp
N_CORES = 8
E = D + 1  # 33: v columns + ones column

# Schraudolph bf16 2^x-trick constants: i16 = round(x*128/ln2 + (127-s)*128),
# bitcast to bf16 ~ exp(x) with ~1.5% mean / ~4% max relative error.  The
# softmax numerator and denominator use the same approximation, so much of
# the error cancels in the ratio.
SCH_A = float(128.0 / np.log(2.0))
SCH_B = float((127.0 - 0.0579) * 128.0)
# Units (of 64 per pair) handled by DVE-Schraudolph instead of ScalarE exp.
# Phase 3 is bound by the cold-clock PE (~1075ns/unit; HAM never sustains
# 2.4GHz under this loop's micro-gaps), so only offload enough exp units
# to bring ScalarE down to the PE floor: 4 per pair.  First 12 units of
# each pair stay on ScalarE so the DVE can drain the previous pair's
# normalize chain without stalling the sc-tile pipeline.
DVE_START = 12
DVE_MOD = (13, 29)  # 2 of each 32-unit sweep


def _use_dve(i):
    return i >= DVE_START and (i % 32) in DVE_MOD


def _body(nc, tc, ctx, x_d, wq_d, wk_d, wv_d, wp_d, bias_d, out_d):
    const = ctx.enter_context(tc.tile_pool(name="const", bufs=1))
    big = ctx.enter_context(tc.tile_pool(name="big", bufs=1))

    ident = const.tile([P, P], F32)
    make_identity(nc, ident)

    wq_sb = const.tile([P, 2, C], BF16)
    wk_sb = const.tile([P, 2, C], BF16)
    wv_sb = const.tile([P, 2, C], BF16)
    wp_sb = const.tile([P, 4, C], BF16)
    bias_sb = const.tile([P, 2], F32)  # biasT: value per c-partition, col=half
    warm = const.tile([P, 1], F32)

    xT = [big.tile([P, T], BF16, name=f"xT{i}") for i in range(2)]
    qT = [big.tile([P, T], BF16, name=f"qT{i}") for i in range(2)]
    kT = [big.tile([P, T], BF16, name=f"kT{i}") for i in range(2)]
    v_sb = big.tile([P, NT, E * H], BF16)
    nc.vector.memset(v_sb, 1.0)  # ones columns survive; v overwrites the rest
    zeros128 = const.tile([P, P], BF16)
    nc.vector.memset(zeros128, 0.0)
    nout = [big.tile([P, T], BF16, name=f"nout{i}") for i in range(4)]
    denb = big.tile([P, T], F32)
    rcp = big.tile([P, T], F32)
    uout = [big.tile([P, T], F32, name=f"uout{i}") for i in range(2)]

    # ---- Phase 1: loads, xT via PE transposes -------------------------
    with tc.tile_pool(name="xload", bufs=1) as xpool, \
         tc.tile_pool(name="pt", bufs=4, space="PSUM") as pt:
        x_sb = xpool.tile([P, NT, C], F32)
        x_r = x_d.rearrange("(n p) c -> p n c", p=P)
        for n in range(NT):  # chunked so transposes start on first arrival;
            # alternate between the two HWDGE queues (SP / Activation)
            eng = nc.sync if n % 2 == 0 else nc.scalar
            eng.dma_start(out=x_sb[:, n], in_=x_r[:, n])
        with tc.tile_pool(name="wstage", bufs=4) as wstage:
            for w_sb, w_d, nk in ((wq_sb, wq_d, 2), (wk_sb, wk_d, 2),
                                  (wv_sb, wv_d, 2), (wp_sb, wp_d, 4)):
                w_st = wstage.tile([P, nk, C], F32, tag="wst", name="w_st")
                if nk == 2:
                    nc.sync.dma_start(
                        out=w_st, in_=w_d.rearrange("(k p) c -> p k c", p=P))
                else:
                    nc.sync.dma_start(out=w_st, in_=w_d.rearrange("q p c -> p q c"))
                nc.vector.tensor_copy(w_sb, w_st)
            nc.sync.dma_start(out=bias_sb, in_=bias_d)
        nc.scalar.activation(out=warm, in_=x_sb[:, 0, 0:1], func=EXP)
        # HAM warmup: ~28 dense zero-weight matmuls while the x DMA streams
        # (PE is idle anyway) flip the PE clock gate to 2.4GHz before the
        # transpose/qkv phase.  Output is scratch, never read.
        wmm = pt.tile([P, 512], F32, tag="wmm", name="wmm")
        for k in range(28):
            nc.tensor.matmul(
                wmm, lhsT=zeros128,
                rhs=v_sb.rearrange("p n e -> p (n e)")[:, 0:512],
                start=True, stop=True)
        for n in range(NT):
            for cc in range(2):
                tp = pt.tile([P, P], F32, tag="tp", name="tp")
                nc.tensor.transpose(tp, x_sb[:, n, cc * P:(cc + 1) * P], ident)
                # copy on ScalarE (idle until the first exp) so the DVE
                # queue is free for the qk casts -- two parallel streams
                nc.scalar.copy(out=xT[cc][:, n * P:(n + 1) * P], in_=tp)

    # ---- Phase 2: qT / kT / v, ordered so pair 0 starts ASAP ----------
    # (scores for units 0..15 need only q-g0 all ts + k-g0 ts0 + v 0..3)
    with tc.tile_pool(name="pq", bufs=4, space="PSUM") as pq, \
         tc.tile_pool(name="pv", bufs=2, space="PSUM") as pv:
        def qk_job(w_sb, dest, g, ts):
            qp = pq.tile([P, 512], F32, tag="qp", name="qp")
            for j in range(4):
                h = 4 * g + j
                for cc in range(2):
                    nc.tensor.matmul(
                        qp[D * j:D * (j + 1), :],
                        lhsT=w_sb[:, cc, D * h:D * (h + 1)],
                        rhs=xT[cc][:, 512 * ts:512 * (ts + 1)],
                        start=(cc == 0), stop=(cc == 1),
                        tile_position=(0, D * j))
            nc.vector.tensor_copy(dest[g][:, 512 * ts:512 * (ts + 1)], qp)

        def v_job(n):
            vp = pv.tile([P, C], F32, tag="vp", name="vp")
            for cc in range(2):
                nc.tensor.matmul(
                    vp,
                    lhsT=xT[cc][:, n * P:(n + 1) * P],
                    rhs=wv_sb[:, cc, :],
                    start=(cc == 0), stop=(cc == 1))
            nc.scalar.copy(
                out=v_sb[:, n].rearrange("p (h e) -> p h e", e=E)[:, :, 0:D],
                in_=vp.rearrange("p (h d) -> p h d", d=D))

        # the first sweep (pair 0, t-half 0) needs q-g0 ts0/1 and all of
        # k-g0 (the s loop spans T); q-g0 ts2/3 only at t-half 1 (~45us in)
        qk_job(wq_sb, qT, 0, 0)
        qk_job(wk_sb, kT, 0, 0)
        qk_job(wq_sb, qT, 0, 1)
        qk_job(wk_sb, kT, 0, 1)
        for n in range(4):
            v_job(n)
        qk_job(wk_sb, kT, 0, 2)
        qk_job(wk_sb, kT, 0, 3)
        for n in range(4, 10):
            v_job(n)
        qk_job(wq_sb, qT, 0, 2)
        qk_job(wq_sb, qT, 0, 3)
        for n in range(10, NT):
            v_job(n)
        for ts in range(4):
            qk_job(wq_sb, qT, 1, ts)
        for ts in range(4):
            qk_job(wk_sb, kT, 1, ts)

    # memsets for normalize-phase tiles: emitted here so they don't delay
    # the phase-1/2 DVE queue (first needed ~95us in, at pair-0 normalize)
    for t_ in nout:
        nc.vector.memset(t_, 0.0)  # rows 33-63 / 97-127 must stay zero
    nc.vector.memset(denb, 1.0)  # rows 32-63 / 96-127 stay 1.0 forever
    nc.vector.memset(rcp, 1.0)  # row 96+ read by the mult, never written
    for t_ in uout:
        nc.vector.memset(t_, 0.0)  # rows 33-63 must stay zero

    # ---- Phase 3: attention, head pairs, 1-unit software pipeline ------
    # Attention runs in 8 sweeps of (head-pair, t-half); av is [128, 1024]
    # (2 banks) and double-buffered, so a sweep's AV accumulation never
    # waits on the previous sweep's normalize chain (no boundary stall).
    with tc.tile_pool(name="scp", bufs=2, space="PSUM") as scp, \
         tc.tile_pool(name="avp", bufs=2, space="PSUM") as avp, \
         tc.tile_pool(name="expp", bufs=6) as expp, \
         tc.tile_pool(name="expi", bufs=4) as expi:
        for pair in range(4):
            g = pair // 2
            hA, hB = 2 * pair, 2 * pair + 1
            jA, jB = hA % 4, hB % 4
            for th in range(2):
                t0 = 1024 * th
                cs = slice(t0, t0 + 1024)
                av = avp.tile([P, 1024], F32, tag="av", name="av")
                units = [(s, tq2) for s in range(NT) for tq2 in range(2)]

                def sc_mms(u):
                    sct = scp.tile([P, 2, 512], F32, tag="sc", name="sc")
                    s, tq2 = u
                    for hi, j in enumerate((jA, jB)):
                        nc.tensor.matmul(
                            sct[:, hi, :],
                            lhsT=kT[g][D * j:D * (j + 1), P * s:P * (s + 1)],
                            rhs=qT[g][D * j:D * (j + 1),
                                      t0 + 512 * tq2:t0 + 512 * (tq2 + 1)],
                            start=True, stop=True,
                            tile_position=(D * j, 0))
                    return sct

                def av_mms(u, ex):
                    s, tq2 = u
                    for hi, h in enumerate((hA, hB)):
                        col = 0 if hi == 0 else 64
                        # A and B share banks at different partition ranges;
                        # per-element has_written makes that safe on HW, but
                        # the sim's bank-granular group tracker would flag it.
                        nc.tensor.matmul(
                            av[col:col + E, 512 * tq2:512 * (tq2 + 1)],
                            lhsT=v_sb[:, s, E * h:E * h + E],
                            rhs=ex[:, hi, :],
                            start=(s == 0), stop=(s == NT - 1),
                            tile_position=(0, col),
                            skip_group_check=True)

                # software pipeline: sc one unit ahead, AV two units behind
                # -- an AV waiting on a DVE-offloaded exp must not block
                # upcoming score MMs (and hence the ACT ring) in the
                # in-order PE queue
                prev = sc_mms(units[0])
                avq = []  # [(unit, ex)] not yet emitted
                for i, u in enumerate(units):
                    nxt = sc_mms(units[i + 1]) if i + 1 < len(units) else None
                    if _use_dve(i):
                        exi_t = expi.tile([P, 2, 512], I16, tag="exi",
                                          name="exi")
                        nc.vector.tensor_scalar(
                            out=exi_t, in0=prev, scalar1=SCH_A, scalar2=SCH_B,
                            op0=mybir.AluOpType.mult, op1=mybir.AluOpType.add)
                        ex = exi_t.bitcast(BF16)
                    else:
                        ex = expp.tile([P, 2, 512], BF16, tag="ex", name="ex")
                        nc.scalar.activation(out=ex, in_=prev, func=EXP)
                    avq.append((u, ex))
                    if len(avq) > 2:
                        av_mms(*avq.pop(0))
                    prev = nxt
                while avq:
                    av_mms(*avq.pop(0))

                # normalize: free av banks (DVE copies), then bcast+recip+
                # mult off the critical path.  reciprocal_approx_fast only
                # works at base partition 0 on HW (single-partition or
                # base-64 APs silently write nothing), so broadcast the RAW
                # denominator rows first and recip [0:96] in one shot (rows
                # 32-63 hold 1.0 -> recip 1.0, harmless).
                uo = uout[pair % 2]
                nc.vector.tensor_copy(uo[0:E, cs], av[0:E, :])
                nc.vector.tensor_copy(uo[64:64 + E, cs], av[64:64 + E, :])
                for row, dst in ((D, 0), (64 + D, 64)):
                    nc.sync.dma_start(
                        out=denb[dst:dst + D, cs],
                        in_=uo[row:row + 1, cs].unsqueeze(1)
                            .broadcast_to((1, D, 1024)))
                nc.vector.reciprocal_approx_fast(
                    out=rcp[0:96, cs], in_=denb[0:96, cs])
                # mult on GpSimd (idle, overlaps next sweep); the last
                # sweep's mult gates the projection -> faster DVE
                eng = nc.vector if pair == 3 else nc.gpsimd
                eng.tensor_tensor(
                    out=nout[pair][0:64 + E, cs], in0=uo[0:64 + E, cs],
                    in1=rcp[0:64 + E, cs], op=mybir.AluOpType.mult)

    # ---- Phase 4: projection TRANSPOSED, M-split 2-way; bias on DVE ----
    with tc.tile_pool(name="prp", bufs=2, space="PSUM") as prp, \
         tc.tile_pool(name="resp", bufs=2) as resp:
        for half in range(2):
            rp = prp.tile([P, T], F32, tag="rp", name="rp")
            for q in range(4):
                for tq in range(4):
                    for m2 in range(2):  # alternate col-groups: 2-way concurrent
                        nc.tensor.matmul(
                            rp[64 * m2:64 * (m2 + 1),
                               512 * tq:512 * (tq + 1)],
                            lhsT=wp_sb[:, q,
                                       P * half + 64 * m2:
                                       P * half + 64 * (m2 + 1)],
                            rhs=nout[q][:, 512 * tq:512 * (tq + 1)],
                            start=(q == 0), stop=(q == 3),
                            tile_position=(0, 64 * m2),
                            skip_group_check=True)
            res = resp.tile([P, T], F32, tag="res", name="res")
            nc.vector.tensor_scalar(
                out=res, in0=rp, scalar1=bias_sb[:, half:half + 1],
                scalar2=None, op0=mybir.AluOpType.add)
            nc.sync.dma_start(out=out_d[half], in_=res)


def build_nc():
    nc = bacc.Bacc("TRN2", debug=False, num_devices=N_CORES)
    x_d = nc.dram_tensor("x", [T, C], F32, kind="ExternalInput")
    wq_d = nc.dram_tensor("wq", [C, C], F32, kind="ExternalInput")
    wk_d = nc.dram_tensor("wk", [C, C], F32, kind="ExternalInput")
    wv_d = nc.dram_tensor("wv", [C, C], F32, kind="ExternalInput")
    wp_d = nc.dram_tensor("wp", [4, P, C], F32, kind="ExternalInput")
    bias_d = nc.dram_tensor("bias", [P, 2], F32, kind="ExternalInput")
    out_d = nc.dram_tensor("out", [2, P, T], F32, kind="ExternalOutput")
    with tile.TileContext(nc) as tc:
        with ExitStack() as ctx:
            _body(nc, tc, ctx, x_d.ap(), wq_d.ap(), wk_d.ap(), wv_d.ap(),
                  wp_d.ap(), bias_d.ap(), out_d.ap())
    nc.compile()
    return nc


def prep_inputs(x, wq, wk, wv, wproj, bproj):
    """Host-side reformatting of the full inputs into per-core input maps."""
    f = np.float32
    # [H,C,D] -> [C, H*D]; wq additionally pre-scaled by 1/sqrt(C) (exact).
    wq2 = np.ascontiguousarray(
        np.transpose(np.asarray(wq, f), (1, 0, 2)).reshape(C, H * D)) * f(1.0 / 16.0)
    wk2 = np.ascontiguousarray(
        np.transpose(np.asarray(wk, f), (1, 0, 2)).reshape(C, H * D))
    wv2 = np.ascontiguousarray(
        np.transpose(np.asarray(wv, f), (1, 0, 2)).reshape(C, H * D))
    # wproj [H*D, C] -> 4 pair-chunks padded to 128 rows:
    # rows 0-31 <- head 2p, rows 64-95 <- head 2p+1, rest zero.
    wp4 = np.zeros((4, P, C), f)
    wproj = np.asarray(wproj, f)
    for p in range(4):
        wp4[p, 0:D] = wproj[64 * p: 64 * p + D]
        wp4[p, 64:64 + D] = wproj[64 * p + D: 64 * p + 2 * D]
    # biasT: [P, 2] -- value per c-partition, column = c-half
    biasT = np.ascontiguousarray(
        np.asarray(bproj, f).reshape(2, P).T)
    x = np.asarray(x, f)
    in_maps = []
    for b in range(N_CORES):
        in_maps.append({
            "x": np.ascontiguousarray(x[b]),
            "wq": wq2, "wk": wk2, "wv": wv2,
            "wp": wp4, "bias": biasT,
        })
    return in_maps


def unshard_out(raw):
    """Per-core raw out [2, 128, T] (c-half, c, t) -> [T, C]."""
    return np.ascontiguousarray(
        np.asarray(raw).reshape(C, T).T)


def kernel(x, wq, wk, wv, wproj, bproj, _nc=None):
    in_maps = prep_inputs(x, wq, wk, wv, wproj, bproj)
    nc = _nc if _nc is not None else build_nc()
    res = run_bass_kernel_spmd(nc, in_maps, list(range(N_CORES)))
    return np.stack([unshard_out(r["out"]) for r in res.results], axis=0)
